# revision 4
# baseline (speedup 1.0000x reference)
"""Criss-cross attention (nn_CC_attention) Trainium2 kernel.

Sharding: pure data parallel over batch B=8 across 8 NeuronCores; the only
cross-core coupling is the global min/max of the energy tensor, exchanged via
a tiny AllReduce(max) of (max, -min).

Per-core algorithm (inputs t1, t2 of shape (C=256, H=128, W=128) fp32):
  phase 1 (streaming over channel groups of 4):
    - DMA fp32 tiles [h,(c,w)] in; cast t2->fp16 (DVE) and t1->fp16 (ACT)
    - key_W[c][h,k] = pool_avg over w of t1 (DVE pool_avg, fp16)
    - key_H[c][w,k] = t1q[c].T @ P  (PE matmul, t1 as weights, P = pool mat /8)
    - t2T[c][w,h]   = PE transpose of t2q[c] (identity matmul) -> fp16 resident
    - energy_W psum[k,w] += key_W[c].T @ t2q[c]   (contract h over all c)
    - energy_H psum[k,h] += key_H[c].T @ t2T[c]   (contract w over all c)
  boundary:
    - local (max,-min) -> AllReduce(max) over 8 cores -> global min/max
    - att = softmax((e-min)/(max-min)) over all 4096 energies (exp on ACT)
    - A_HT[h',h] = 0.0625*att_H[h,h'//8] + 0.5*I   (PE expand + DVE +I)
    - M_W[w',w]  = 0.0625*att_W[w,w'//8] + 0.5*I
  phase 2 (per channel group):
    - psum[h,(c,w)] = A_HT.T @ t2q  (+= per c: t2T[c].T @ M_W)
      = 0.5*out_H + 0.5*out_W + t2   (the +t2 rides the 0.5*I in both mats)
    - ACT copy psum->sbuf fp32, DMA out.
"""

import numpy as np
from contextlib import ExitStack

import concourse.bass as bass
import concourse.tile as tile
from concourse import bacc, mybir

B, C, H, W, POOL = 8, 256, 128, 128, 8
KH, KW = H // POOL, W // POOL  # 16, 16
NCORES = 8
G = 4  # channels per group tile

F32 = mybir.dt.float32
F16 = mybir.dt.float16


def host_constants(dtype_np=np.float16):
    """Constant matrices shipped as extra kernel inputs."""
    pool16 = np.zeros((H, KH), np.float32)
    for k in range(KH):
        pool16[k * POOL:(k + 1) * POOL, k] = 1.0 / POOL
    ident16 = np.eye(H, dtype=np.float32)
    expmat = np.zeros((KH, H), np.float32)
    for k in range(KH):
        expmat[k, k * POOL:(k + 1) * POOL] = 0.5 / POOL  # 0.0625
    eyehalf = 0.5 * np.eye(H, dtype=np.float32)
    return {
        "pool16": pool16.astype(dtype_np),
        "ident16": ident16.astype(dtype_np),
        "expmat": expmat.astype(np.float32),
        "eyehalf": eyehalf.astype(np.float32),
    }


def build(c_total=C, ncores=NCORES):
    assert c_total % G == 0
    ngroups = c_total // G
    nc = bacc.Bacc(trn_type="TRN2", target_bir_lowering=False, debug=False,
                   num_devices=ncores)

    t1 = nc.dram_tensor("t1", [c_total, H, W], F32, kind="ExternalInput").ap()
    t2 = nc.dram_tensor("t2", [c_total, H, W], F32, kind="ExternalInput").ap()
    pool16 = nc.dram_tensor("pool16", [H, KH], F16, kind="ExternalInput").ap()
    ident16 = nc.dram_tensor("ident16", [H, W], F16, kind="ExternalInput").ap()
    expmat = nc.dram_tensor("expmat", [KH, H], F32, kind="ExternalInput").ap()
    eyehalf = nc.dram_tensor("eyehalf", [H, W], F32, kind="ExternalInput").ap()
    out = nc.dram_tensor("out", [c_total, H, W], F32, kind="ExternalOutput").ap()

    with tile.TileContext(nc) as tc, ExitStack() as top:
        # ---- constants ----
        cpool = top.enter_context(tc.tile_pool(name="consts", bufs=1))
        c_pool16 = cpool.tile([H, KH], F16, tag="pool16")
        nc.sync.dma_start(c_pool16[:], pool16[:])
        c_ident = cpool.tile([H, W], F16, tag="ident16")
        nc.sync.dma_start(c_ident[:], ident16[:])
        c_expmat = cpool.tile([KH, H], F32, tag="expmat")
        nc.sync.dma_start(c_expmat[:], expmat[:])
        c_eyehalf = cpool.tile([H, W], F32, tag="eyehalf")
        nc.sync.dma_start(c_eyehalf[:], eyehalf[:])

        # ---- resident fp16 copies of t2 (natural + transposed) ----
        resq = top.enter_context(tc.tile_pool(name="resq", bufs=ngroups))
        resqT = top.enter_context(tc.tile_pool(name="resqT", bufs=ngroups))
        t2q_tiles = []
        t2Tq_tiles = []

        # ---- persistent energy accumulators ----
        ps_e = top.enter_context(tc.tile_pool(name="ps_e", bufs=1, space="PSUM"))
        ps_eW = ps_e.tile([KH, W], F32, tag="eW")
        ps_eH = ps_e.tile([KH, H], F32, tag="eH")

        # ---- softmax / stats pool (outer; small) ----
        spool = top.enter_context(tc.tile_pool(name="soft", bufs=1))
        dram = top.enter_context(tc.tile_pool(name="dram", bufs=1, space="DRAM"))

        # ================= phase 1 =================
        with ExitStack() as ph1:
            pin = ph1.enter_context(tc.tile_pool(name="pin", bufs=3))
            qpool = ph1.enter_context(tc.tile_pool(name="t1q", bufs=3))
            kpool = ph1.enter_context(tc.tile_pool(name="keys", bufs=3))
            ps_kh = ph1.enter_context(tc.tile_pool(name="ps_kh", bufs=2, space="PSUM"))
            ps_tT = ph1.enter_context(tc.tile_pool(name="ps_tT", bufs=2, space="PSUM"))

            for g in range(ngroups):
                c0 = g * G
                t1g = pin.tile([H, G * W], F32, tag="t1g")
                nc.sync.dma_start(t1g[:].rearrange("p (c w) -> p c w", c=G),
                                  t1[c0:c0 + G].rearrange("c h w -> h c w"))
                t2g = pin.tile([H, G * W], F32, tag="t2g")
                nc.sync.dma_start(t2g[:].rearrange("p (c w) -> p c w", c=G),
                                  t2[c0:c0 + G].rearrange("c h w -> h c w"))

                # fp16 casts
                t2q = resq.tile([H, G * W], F16, tag="t2q")
                nc.vector.tensor_copy(t2q[:], t2g[:])
                t2q_tiles.append(t2q)
                t1q = qpool.tile([H, G * W], F16, tag="t1q")
                nc.scalar.copy(t1q[:], t1g[:])

                # key_W[c][h,k]: sum-pool over w (innermost 8) on DVE, then /8 + cast
                kWr = kpool.tile([H, G * KW], F32, tag="kWr")
                nc.vector.tensor_reduce(
                    kWr[:].rearrange("p (c k) -> p c k", c=G),
                    t1g[:].rearrange("p (c k j) -> p c k j", c=G, j=POOL),
                    axis=mybir.AxisListType.X, op=mybir.AluOpType.add)
                kW = kpool.tile([H, G * KW], F16, tag="kW")
                nc.vector.tensor_scalar_mul(kW[:], kWr[:], 1.0 / POOL)

                # key_H[c][w,k] = t1q[c].T @ pool16  (t1q as stationary weights)
                ps_kh_t = ps_kh.tile([W, G * KH], F32, tag="ps_kh")
                for i in range(G):
                    nc.tensor.matmul(ps_kh_t[:, i * KH:(i + 1) * KH],
                                     t1q[:, i * W:(i + 1) * W], c_pool16[:],
                                     start=True, stop=True)
                kH = kpool.tile([W, G * KH], F16, tag="kH")
                nc.scalar.copy(kH[:], ps_kh_t[:])

                # t2T[c][w,h] via PE transpose
                ps_tT_t = ps_tT.tile([W, G * H], F16, tag="ps_tT")
                for i in range(G):
                    nc.tensor.transpose(ps_tT_t[:, i * H:(i + 1) * H],
                                        t2q[:, i * W:(i + 1) * W], c_ident[:])
                t2Tq = resqT.tile([W, G * H], F16, tag="t2Tq")
                nc.scalar.copy(t2Tq[:], ps_tT_t[:])
                t2Tq_tiles.append(t2Tq)

                # energy accumulation
                first = (g == 0)
                last = (g == ngroups - 1)
                for i in range(G):
                    nc.tensor.matmul(ps_eW[:], kW[:, i * KW:(i + 1) * KW],
                                     t2q[:, i * W:(i + 1) * W],
                                     start=(first and i == 0), stop=(last and i == G - 1))
                    nc.tensor.matmul(ps_eH[:], kH[:, i * KH:(i + 1) * KH],
                                     t2Tq[:, i * H:(i + 1) * H],
                                     start=(first and i == 0), stop=(last and i == G - 1))

        # ================= boundary: softmax + collective =================
        e_sb = spool.tile([KH, H + W], F32, tag="e_sb")
        nc.vector.tensor_copy(e_sb[:, 0:H], ps_eH[:])
        nc.vector.tensor_copy(e_sb[:, H:H + W], ps_eW[:])

        # local min/max
        pack = spool.tile([KH, 2], F32, tag="pack")
        nc.vector.tensor_reduce(pack[:, 0:1], e_sb[:], axis=mybir.AxisListType.X,
                                op=mybir.AluOpType.max)
        rmin = spool.tile([KH, 1], F32, tag="rmin")
        nc.vector.tensor_reduce(rmin[:], e_sb[:], axis=mybir.AxisListType.X,
                                op=mybir.AluOpType.min)
        nc.vector.tensor_scalar_mul(pack[:, 1:2], rmin[:], -1.0)
        row = spool.tile([1, 2 * KH], F32, tag="row")
        nc.sync.dma_start(row[:], pack[:])  # [16,2] -> [1,32] (partition-major)
        l2 = spool.tile([1, 2], F32, tag="l2")
        nc.vector.tensor_reduce(l2[:], row[:].rearrange("a (s t) -> a t s", t=2),
                                axis=mybir.AxisListType.X, op=mybir.AluOpType.max)

        # collective: AllReduce(max) of padded [1,128]
        cbuf = spool.tile([1, 128], F32, tag="cbuf")
        nc.vector.memset(cbuf[:], -3.0e38)
        nc.vector.tensor_copy(cbuf[:, 0:2], l2[:])
        cc_in = dram.tile([1, 128], F32, tag="cc_in")
        cc_out = dram.tile([1, 128], F32, tag="cc_out")
        nc.sync.dma_start(cc_in[:], cbuf[:])
        nc.gpsimd.collective_compute(
            "AllReduce", mybir.AluOpType.max,
            replica_groups=[list(range(ncores))],
            ins=[cc_in.opt()], outs=[cc_out.opt()],
        )
        g2 = spool.tile([1, 2], F32, tag="g2")
        nc.sync.dma_start(g2[:], cc_out[:, 0:2])

        # inv = 1/(gmax - gmin); bias = -gmin/(gmax-gmin)
        rng_t = spool.tile([1, 1], F32, tag="rng")
        nc.vector.tensor_tensor(rng_t[:], g2[:, 0:1], g2[:, 1:2], mybir.AluOpType.add)
        inv_t = spool.tile([1, 1], F32, tag="inv")
        nc.vector.reciprocal(inv_t[:], rng_t[:])
        sb_t = spool.tile([1, 2], F32, tag="sb")
        nc.vector.tensor_copy(sb_t[:, 0:1], inv_t[:])
        nc.vector.tensor_tensor(sb_t[:, 1:2], g2[:, 1:2], inv_t[:], mybir.AluOpType.mult)
        sc16 = spool.tile([KH, 2], F32, tag="sc16")
        nc.gpsimd.partition_broadcast(sc16[:], sb_t[:])

        # exp((e - gmin)/range) with per-row sums
        s_sb = spool.tile([KH, H + W], F32, tag="s_sb")
        ssum = spool.tile([KH, 1], F32, tag="ssum")
        nc.scalar.activation(s_sb[:], e_sb[:], mybir.ActivationFunctionType.Exp,
                             bias=sc16[:, 1:2], scale=sc16[:, 0:1], accum_out=ssum[:])
        srow = spool.tile([1, KH], F32, tag="srow")
        nc.sync.dma_start(srow[:], ssum[:])
        stot = spool.tile([1, 1], F32, tag="stot")
        nc.vector.tensor_reduce(stot[:], srow[:], axis=mybir.AxisListType.X,
                                op=mybir.AluOpType.add)
        rnorm = spool.tile([1, 1], F32, tag="rnorm")
        nc.vector.reciprocal(rnorm[:], stot[:])
        rn16 = spool.tile([KH, 1], F32, tag="rn16")
        nc.gpsimd.partition_broadcast(rn16[:], rnorm[:])
        s_att = spool.tile([KH, H + W], F32, tag="s_att")
        nc.vector.tensor_scalar_mul(s_att[:], s_sb[:], rn16[:])

        # A_HT / M_W build
        apool = top.enter_context(tc.tile_pool(name="amats", bufs=1))
        with tc.tile_pool(name="ps_a", bufs=2, space="PSUM") as ps_a:
            ps_ah = ps_a.tile([H, H], F32, tag="ps_ah")
            nc.tensor.matmul(ps_ah[:], c_expmat[:], s_att[:, 0:H], start=True, stop=True)
            A_HT = apool.tile([H, H], F16, tag="A_HT")
            nc.vector.scalar_tensor_tensor(A_HT[:], ps_ah[:], 1.0, c_eyehalf[:],
                                           op0=mybir.AluOpType.mult,
                                           op1=mybir.AluOpType.add)
            ps_mw = ps_a.tile([W, W], F32, tag="ps_mw")
            nc.tensor.matmul(ps_mw[:], c_expmat[:], s_att[:, H:H + W], start=True, stop=True)
            M_W = apool.tile([W, W], F16, tag="M_W")
            nc.vector.scalar_tensor_tensor(M_W[:], ps_mw[:], 1.0, c_eyehalf[:],
                                           op0=mybir.AluOpType.mult,
                                           op1=mybir.AluOpType.add)

        # ================= phase 2 =================
        with ExitStack() as ph2:
            ps_out = ph2.enter_context(tc.tile_pool(name="ps_out", bufs=4, space="PSUM"))
            opool = ph2.enter_context(tc.tile_pool(name="outp", bufs=3))
            for g in range(ngroups):
                c0 = g * G
                ps_o = ps_out.tile([H, G * W], F32, tag="ps_o")
                nc.tensor.matmul(ps_o[:], A_HT[:], t2q_tiles[g][:], start=True, stop=False)
                for i in range(G):
                    nc.tensor.matmul(ps_o[:, i * W:(i + 1) * W],
                                     t2Tq_tiles[g][:, i * H:(i + 1) * H], M_W[:],
                                     start=False, stop=(i == G - 1))
                ob = opool.tile([H, G * W], F32, tag="ob")
                nc.scalar.copy(ob[:], ps_o[:])
                nc.sync.dma_start(out[c0:c0 + G].rearrange("c h w -> h c w"),
                                  ob[:].rearrange("p (c w) -> p c w", c=G))

    nc.compile()
    return nc


_NC_CACHE = {}


def _get_nc():
    key = (C, NCORES)
    if key not in _NC_CACHE:
        _NC_CACHE[key] = build(C, NCORES)
    return _NC_CACHE[key]


def kernel(tensor1: np.ndarray, tensor2: np.ndarray) -> np.ndarray:
    from concourse.bass_utils import run_bass_kernel_spmd
    assert tensor1.shape == (B, C, H, W) and tensor2.shape == (B, C, H, W)
    nc = _get_nc()
    consts = host_constants()
    in_maps = [
        {"t1": np.ascontiguousarray(tensor1[b], dtype=np.float32),
         "t2": np.ascontiguousarray(tensor2[b], dtype=np.float32),
         **consts}
        for b in range(B)
    ]
    res = run_bass_kernel_spmd(nc, in_maps, core_ids=list(range(NCORES)))
    return np.stack([res.results[b]["out"] for b in range(B)]).astype(np.float32)


# revision 7
# speedup vs baseline: 1.6410x; 1.6410x over previous
"""Criss-cross attention (nn_CC_attention) Trainium2 kernel.

Sharding: pure data parallel over batch B=8 across 8 NeuronCores; the only
cross-core coupling is the global min/max of energy, exchanged via a tiny
AllReduce(max) of (max, -min).

Host-side staging (layout/precision only; all model compute is on-device):
  t1b = bf16(tensor1) as (H, C, W)   -- keys source
  t2h = fp16(tensor2) as (H, C, W)   -- carries the exact +tensor2 path
  t2t = bf16(tensor2) as (W, C, H)   -- pre-transposed copy for the W branch
  out is produced as fp16 (H, C, W), host transposes back to (C, H, W) fp32.

Per-core device algorithm:
  phase 1 (stream 32 groups of 8 channels):
    kW[c][h,k] = avg-pool_w(t1)  (DVE reduce + scale->fp16)
    kH[c][w,k] = t1b[c].T @ P    (PE, t1 as stationary; P = pooling matrix /8)
    eW[w,k] += t2h[c].T @ kW[c]  (PE, t2h stationary, K=h)
    eH[h,k] += t2t[c].T @ kH[c]  (PE, t2t stationary, K=w)
  boundary:
    local (max,-min) -> AllReduce(max) -> global range; exp on ACT; sums;
    att -> A_HT[h',h] = 0.0625*att_H[h,h'//8] + I (fp16)
           M_W[w',w]  = 0.0625*att_W[w,w'//8]     (bf16)
    (0.5 gamma is folded into the 0.0625; the full +tensor2 rides A_HT's I)
  phase 2 (per group):
    psum[h,(c,w)] = A_HT.T @ t2h[g]  (N=512 x2)
                  += t2t[c].T @ M_W  (per c)
    = 0.5*out_H + 0.5*out_W + tensor2 ;  ACT copy -> fp16 -> DMA out.
PE warm-up bursts (dummy matmuls) run at kernel start and during the
collective so the HAM clock gate is open (2.4 GHz) for both phases.
"""

import numpy as np
from contextlib import ExitStack

import ml_dtypes
import concourse.bass as bass
import concourse.tile as tile
from concourse import bacc, mybir

B, C, H, W, POOL = 8, 256, 128, 128, 8
KH, KW = H // POOL, W // POOL  # 16, 16
NCORES = 8
G = 8  # channels per group tile
NWARM = 56  # dummy matmuls per PE warm-up burst

F32 = mybir.dt.float32
F16 = mybir.dt.float16
BF16 = mybir.dt.bfloat16
BF_NP = ml_dtypes.bfloat16


def host_constants():
    pool_m = np.zeros((H, KH), np.float32)
    for k in range(KH):
        pool_m[k * POOL:(k + 1) * POOL, k] = 1.0 / POOL
    expmat = np.zeros((KH, H), np.float32)
    for k in range(KH):
        expmat[k, k * POOL:(k + 1) * POOL] = 0.5 / POOL  # 0.0625
    return {
        "pool16": pool_m.astype(BF_NP),
        "ident16": np.eye(H, dtype=np.float32).astype(BF_NP),
        "expmat": expmat.astype(BF_NP),
        "eyefull": np.eye(H, dtype=np.float32),
    }


def build(c_total=C, ncores=NCORES):
    assert c_total % G == 0
    ngroups = c_total // G
    nc = bacc.Bacc(trn_type="TRN2", target_bir_lowering=False, debug=False,
                   num_devices=ncores)

    t1b = nc.dram_tensor("t1b", [H, c_total, W], BF16, kind="ExternalInput").ap()
    t2h = nc.dram_tensor("t2h", [H, c_total, W], F16, kind="ExternalInput").ap()
    t2t = nc.dram_tensor("t2t", [W, c_total, H], BF16, kind="ExternalInput").ap()
    pool16 = nc.dram_tensor("pool16", [H, KH], BF16, kind="ExternalInput").ap()
    ident16 = nc.dram_tensor("ident16", [H, W], BF16, kind="ExternalInput").ap()
    expmat = nc.dram_tensor("expmat", [KH, H], BF16, kind="ExternalInput").ap()
    eyefull = nc.dram_tensor("eyefull", [H, W], F32, kind="ExternalInput").ap()
    out = nc.dram_tensor("out", [H, c_total, W], F16, kind="ExternalOutput").ap()

    with tile.TileContext(nc) as tc, ExitStack() as top:
        # ---- constants ----
        cpool = top.enter_context(tc.tile_pool(name="consts", bufs=1))
        c_pool16 = cpool.tile([H, KH], BF16, tag="pool16")
        nc.sync.dma_start(c_pool16[:], pool16[:])
        c_ident = cpool.tile([H, W], BF16, tag="ident16")
        nc.sync.dma_start(c_ident[:], ident16[:])
        c_expmat = cpool.tile([KH, H], BF16, tag="expmat")
        nc.sync.dma_start(c_expmat[:], expmat[:])
        c_eye = cpool.tile([H, W], F32, tag="eyefull")
        nc.sync.dma_start(c_eye[:], eyefull[:])

        resq = top.enter_context(tc.tile_pool(name="resq", bufs=ngroups))
        resqT = top.enter_context(tc.tile_pool(name="resqT", bufs=ngroups))
        t2q_tiles, t2t_tiles = [], []

        psb = ExitStack()  # psum pools released before phase 2
        ps_e = psb.enter_context(tc.tile_pool(name="ps_e", bufs=1, space="PSUM"))
        ps_eW = ps_e.tile([W, KW], F32, tag="eW")
        ps_eH = ps_e.tile([H, KH], F32, tag="eH")
        ps_warm = psb.enter_context(tc.tile_pool(name="ps_warm", bufs=1, space="PSUM"))
        ps_w = ps_warm.tile([H, W], F32, tag="warm")

        spool = top.enter_context(tc.tile_pool(name="soft", bufs=1))
        dram = top.enter_context(tc.tile_pool(name="dram", bufs=1, space="DRAM"))

        # PE warm-up burst #1 (no data deps -> scheduled at kernel start)
        for _ in range(NWARM):
            nc.tensor.matmul(ps_w[:], c_ident[:], c_ident[:], start=True, stop=True)

        # ================= phase 1 =================
        with ExitStack() as ph1:
            pin = ph1.enter_context(tc.tile_pool(name="pin", bufs=3))
            kpool = ph1.enter_context(tc.tile_pool(name="keys", bufs=3))
            ps_kh = ph1.enter_context(tc.tile_pool(name="ps_kh", bufs=2, space="PSUM"))

            for g in range(ngroups):
                c0 = g * G
                t1g = pin.tile([H, G * W], BF16, tag="t1g")
                nc.sync.dma_start(t1g[:].rearrange("p (c w) -> p c w", c=G),
                                  t1b[:, c0:c0 + G, :])
                t2qg = resq.tile([H, G * W], F16, tag="t2qg")
                nc.sync.dma_start(t2qg[:].rearrange("p (c w) -> p c w", c=G),
                                  t2h[:, c0:c0 + G, :])
                t2q_tiles.append(t2qg)
                t2tg = resqT.tile([W, G * H], BF16, tag="t2tg")
                nc.sync.dma_start(t2tg[:].rearrange("p (c h) -> p c h", c=G),
                                  t2t[:, c0:c0 + G, :])
                t2t_tiles.append(t2tg)

                # kW[c][h,k] (fp16)
                kWr = kpool.tile([H, G * KW], F32, tag="kWr")
                nc.vector.tensor_reduce(
                    kWr[:].rearrange("p (c k) -> p c k", c=G),
                    t1g[:].rearrange("p (c k j) -> p c k j", c=G, j=POOL),
                    axis=mybir.AxisListType.X, op=mybir.AluOpType.add)
                kW = kpool.tile([H, G * KW], F16, tag="kW")
                nc.vector.tensor_scalar_mul(kW[:], kWr[:], 1.0 / POOL)

                # kH[c][w,k] (bf16) = t1b[c].T @ pool16
                ps_kh_t = ps_kh.tile([W, G * KH], F32, tag="ps_kh")
                for i in range(G):
                    nc.tensor.matmul(ps_kh_t[:, i * KH:(i + 1) * KH],
                                     t1g[:, i * W:(i + 1) * W], c_pool16[:],
                                     start=True, stop=True)
                kH = kpool.tile([W, G * KH], BF16, tag="kH")
                nc.scalar.copy(kH[:], ps_kh_t[:])

                first = (g == 0)
                last = (g == ngroups - 1)
                for i in range(G):
                    # eW[w,k] += t2h[c].T @ kW[c]
                    nc.tensor.matmul(ps_eW[:], t2qg[:, i * W:(i + 1) * W],
                                     kW[:, i * KW:(i + 1) * KW],
                                     start=(first and i == 0), stop=(last and i == G - 1))
                    # eH[h,k] += t2t[c].T @ kH[c]
                    nc.tensor.matmul(ps_eH[:], t2tg[:, i * H:(i + 1) * H],
                                     kH[:, i * KH:(i + 1) * KH],
                                     start=(first and i == 0), stop=(last and i == G - 1))

        # ================= boundary =================
        e_sb = spool.tile([H, 2 * KH], F32, tag="e_sb")
        nc.vector.tensor_copy(e_sb[:, 0:KH], ps_eH[:])
        nc.vector.tensor_copy(e_sb[:, KH:2 * KH], ps_eW[:])

        # PE warm-up burst #2: depends on e_sb -> runs during the collective
        wtrig = spool.tile([H, 1], BF16, tag="wtrig")
        nc.vector.tensor_copy(wtrig[:], e_sb[:, 0:1])
        for _ in range(NWARM):
            nc.tensor.matmul(ps_w[:, 0:1], c_ident[:], wtrig[:], start=True, stop=True)

        pack = spool.tile([H, 2], F32, tag="pack")
        nc.vector.tensor_reduce(pack[:, 0:1], e_sb[:], axis=mybir.AxisListType.X,
                                op=mybir.AluOpType.max)
        rmin = spool.tile([H, 1], F32, tag="rmin")
        nc.vector.tensor_reduce(rmin[:], e_sb[:], axis=mybir.AxisListType.X,
                                op=mybir.AluOpType.min)
        nc.vector.tensor_scalar_mul(pack[:, 1:2], rmin[:], -1.0)
        row = spool.tile([1, 2 * H], F32, tag="row")
        nc.sync.dma_start(row[:], pack[:])  # [128,2] -> [1,256]
        l2 = spool.tile([1, 2], F32, tag="l2")
        nc.vector.tensor_reduce(l2[:], row[:].rearrange("a (s t) -> a t s", t=2),
                                axis=mybir.AxisListType.X, op=mybir.AluOpType.max)

        cbuf = spool.tile([1, 128], F32, tag="cbuf")
        nc.vector.memset(cbuf[:], -3.0e38)
        nc.vector.tensor_copy(cbuf[:, 0:2], l2[:])
        cc_in = dram.tile([1, 128], F32, tag="cc_in")
        cc_out = dram.tile([1, 128], F32, tag="cc_out")
        nc.sync.dma_start(cc_in[:], cbuf[:])
        nc.gpsimd.collective_compute(
            "AllReduce", mybir.AluOpType.max,
            replica_groups=[list(range(ncores))],
            ins=[cc_in.opt()], outs=[cc_out.opt()],
        )
        g2 = spool.tile([1, 2], F32, tag="g2")
        nc.sync.dma_start(g2[:], cc_out[:, 0:2])

        rng_t = spool.tile([1, 1], F32, tag="rng")
        nc.vector.tensor_tensor(rng_t[:], g2[:, 0:1], g2[:, 1:2], mybir.AluOpType.add)
        inv_t = spool.tile([1, 1], F32, tag="inv")
        nc.vector.reciprocal(inv_t[:], rng_t[:])
        sb_t = spool.tile([1, 2], F32, tag="sb")
        nc.vector.tensor_copy(sb_t[:, 0:1], inv_t[:])
        nc.vector.tensor_tensor(sb_t[:, 1:2], g2[:, 1:2], inv_t[:], mybir.AluOpType.mult)
        sc2 = spool.tile([H, 2], F32, tag="sc2")
        nc.gpsimd.partition_broadcast(sc2[:], sb_t[:])

        s_sb = spool.tile([H, 2 * KH], F32, tag="s_sb")
        ssum = spool.tile([H, 1], F32, tag="ssum")
        nc.scalar.activation(s_sb[:], e_sb[:], mybir.ActivationFunctionType.Exp,
                             bias=sc2[:, 1:2], scale=sc2[:, 0:1], accum_out=ssum[:])
        srow = spool.tile([1, H], F32, tag="srow")
        nc.sync.dma_start(srow[:], ssum[:])
        stot = spool.tile([1, 1], F32, tag="stot")
        nc.vector.tensor_reduce(stot[:], srow[:], axis=mybir.AxisListType.X,
                                op=mybir.AluOpType.add)
        rnorm = spool.tile([1, 1], F32, tag="rnorm")
        nc.vector.reciprocal(rnorm[:], stot[:])
        rn = spool.tile([H, 1], F32, tag="rn")
        nc.gpsimd.partition_broadcast(rn[:], rnorm[:])
        s16 = spool.tile([H, 2 * KH], BF16, tag="s16")
        nc.vector.tensor_scalar_mul(s16[:], s_sb[:], rn[:])

        # att transposes + A-mat builds
        apool = top.enter_context(tc.tile_pool(name="amats", bufs=1))
        with tc.tile_pool(name="ps_a", bufs=1, space="PSUM") as ps_a:
            ps_tH = ps_a.tile([KH, H], BF16, tag="ps_tH")
            nc.tensor.transpose(ps_tH[:], s16[:, 0:KH], c_ident[:])
            att_kh = spool.tile([KH, H], BF16, tag="att_kh")
            nc.scalar.copy(att_kh[:], ps_tH[:])
            ps_tW = ps_a.tile([KH, W], BF16, tag="ps_tW")
            nc.tensor.transpose(ps_tW[:], s16[:, KH:2 * KH], c_ident[:])
            att_kw = spool.tile([KH, W], BF16, tag="att_kw")
            nc.scalar.copy(att_kw[:], ps_tW[:])

            ps_ah = ps_a.tile([H, H], F32, tag="ps_ah")
            nc.tensor.matmul(ps_ah[:], c_expmat[:], att_kh[:], start=True, stop=True)
            A_HT = apool.tile([H, H], F16, tag="A_HT")
            nc.vector.scalar_tensor_tensor(A_HT[:], ps_ah[:], 1.0, c_eye[:],
                                           op0=mybir.AluOpType.mult,
                                           op1=mybir.AluOpType.add)
            ps_mw = ps_a.tile([W, W], F32, tag="ps_mw")
            nc.tensor.matmul(ps_mw[:], c_expmat[:], att_kw[:], start=True, stop=True)
            M_W = apool.tile([W, W], BF16, tag="M_W")
            nc.scalar.copy(M_W[:], ps_mw[:])

        psb.close()

        # ================= phase 2 =================
        with ExitStack() as ph2:
            ps_out = ph2.enter_context(tc.tile_pool(name="ps_out", bufs=3, space="PSUM"))
            opool = ph2.enter_context(tc.tile_pool(name="outp", bufs=3))
            for g in range(ngroups):
                c0 = g * G
                t2qg, t2tg = t2q_tiles[g], t2t_tiles[g]
                ps_o = ps_out.tile([H, G * W], F32, tag="ps_o")
                nc.tensor.matmul(ps_o[:, 0:512], A_HT[:], t2qg[:, 0:512],
                                 start=True, stop=False)
                nc.tensor.matmul(ps_o[:, 512:1024], A_HT[:], t2qg[:, 512:1024],
                                 start=True, stop=False)
                for i in range(G):
                    nc.tensor.matmul(ps_o[:, i * W:(i + 1) * W],
                                     t2tg[:, i * H:(i + 1) * H], M_W[:],
                                     start=False, stop=(i % 4 == 3))
                ob = opool.tile([H, G * W], F16, tag="ob")
                nc.scalar.copy(ob[:], ps_o[:])
                nc.sync.dma_start(out[:, c0:c0 + G, :],
                                  ob[:].rearrange("p (c w) -> p c w", c=G))

    nc.compile()
    return nc


_NC_CACHE = {}


def _get_nc():
    key = (C, NCORES)
    if key not in _NC_CACHE:
        _NC_CACHE[key] = build(C, NCORES)
    return _NC_CACHE[key]


def _stage(tensor1, tensor2):
    """Host-side precision/layout staging for all cores."""
    t1b = np.ascontiguousarray(
        tensor1.astype(BF_NP).transpose(0, 2, 1, 3))            # (B,H,C,W) bf16
    t2h = np.ascontiguousarray(
        tensor2.astype(np.float16).transpose(0, 2, 1, 3))       # (B,H,C,W) fp16
    t2t = np.ascontiguousarray(
        tensor2.astype(BF_NP).transpose(0, 3, 1, 2))            # (B,W,C,H) bf16
    return t1b, t2h, t2t


def kernel(tensor1: np.ndarray, tensor2: np.ndarray) -> np.ndarray:
    from concourse.bass_utils import run_bass_kernel_spmd
    assert tensor1.shape == (B, C, H, W) and tensor2.shape == (B, C, H, W)
    nc = _get_nc()
    consts = host_constants()
    t1b, t2h, t2t = _stage(np.asarray(tensor1, np.float32),
                           np.asarray(tensor2, np.float32))
    in_maps = [
        {"t1b": t1b[b], "t2h": t2h[b], "t2t": t2t[b], **consts}
        for b in range(B)
    ]
    res = run_bass_kernel_spmd(nc, in_maps, core_ids=list(range(NCORES)))
    out_hcw = np.stack([res.results[b]["out"] for b in range(B)])  # (B,H,C,W) f16
    return np.ascontiguousarray(
        out_hcw.transpose(0, 2, 1, 3).astype(np.float32))
